# revision 14
# baseline (speedup 1.0000x reference)
"""RWKV-4 WKV attention layer on 8 TRN2 NeuronCores.

Reference computation (T=4096, NE=DA=2048, fp32):
    xx  = shift(x)  (zero-pad first row)
    xk/xv/xr = lerp(xx, x, time_mix_*)
    k, v, r = xk @ Wk, xv @ Wv, xr @ Wr
    wkv = serial scan over T with per-channel decay lam = exp(-exp(time_decay)),
          bonus eu = exp(time_first)
    out = (sigmoid(r) * wkv) @ Wo

Distribution strategy (v3 — T-sharded scan, single tiny collective):
  Core i owns tokens [512i, 512(i+1)).  It computes k/v/r for ALL 2048
  channels of its own tokens (activations moving, weights stationary),
  runs the unstabilized linear WKV recurrence LOCALLY over its 512
  tokens (per-channel scans on DVE), and the only cross-core traffic is
  the per-channel scan carry: each core's end state (P,Q)[2048] is
  AllGathered ([128,32] f32 per core -> 128KB total), every core
  prefix-combines the 8 carries locally (scan over the core axis with
  factor lam^512), selects its predecessor's prefix via a Switch on the
  partition id, and folds it into its local scan with a host-precomputed
  lambda^n table:  num[c,n] = numA[c,n] + lam_c^n * P_prev[c]  (one
  fused scalar_tensor_tensor per tile).  wkv emerges token-sharded,
  which is exactly what the token-sharded output matmul needs — the
  k/v AllToAll exchanges and the wkv exchange of v2 are gone (6 big
  collectives -> 1 small one).
  r stays token-sharded in fp8 DoubleRow (2x PE) with a fused Sigmoid
  drain, as in v2.  xv/xr are derived from xk (xv = xk + (tmv-tmk)*d)
  so the transposed-x chunks free right after the k-mixes.
"""

import math
import os
import sys
from contextlib import ExitStack

for _p in ("/opt/trn_rl_repo", "/root/.axon_site/_ro/trn_rl_repo"):
    if os.path.isdir(_p) and _p not in sys.path:
        sys.path.insert(0, _p)

import numpy as np
import ml_dtypes

import concourse.bass as bass
import concourse.tile as tile
from concourse import bacc, mybir
from concourse.bass_utils import run_bass_kernel_spmd

F32 = mybir.dt.float32
BF16 = mybir.dt.bfloat16
F8 = mybir.dt.float8e4
AL = mybir.AluOpType
ACTF = mybir.ActivationFunctionType
P = 128

# r-projection in fp8 e4m3 with DoubleRow (2x PE throughput).  Host scales
# W_receptance by 2^5 before quantization; the sigmoid drain divides it out.
FP8_R = True
WR_SCALE = 32.0


class Cfg:
    def __init__(self, T=4096, NE=2048, DA=2048, NC=8):
        self.T, self.NE, self.DA, self.NC = T, NE, DA, NC
        self.TSL = T // NC          # tokens per core
        self.NKT = NE // P          # contraction ptiles (projections)
        self.NKT2 = DA // P         # channel ptiles (= scan ptiles)
        self.NMT = self.TSL // P    # T ptiles per slice
        self.NOT = NE // 512        # N tiles (output matmul)
        assert self.TSL % P == 0 and DA % 512 == 0 and NE % 512 == 0


def _bcast(ap, n):
    """[P,1] AP -> [P,n] stride-0 broadcast along free."""
    return bass.AP(ap.tensor, ap.offset, [ap.ap[0], [0, n]])


def build_kernel(cfg: Cfg, no_cc: bool = False, reps: int = 1,
                 cc_copy: bool = False, ablate: str | None = None):
    nc = bacc.Bacc("TRN2", target_bir_lowering=False, debug=False,
                   num_devices=1 if no_cc else cfg.NC)

    def _collective(kind, op, replica_groups, ins, outs, cc_tiles=None):
        if no_cc or cc_copy:
            # timing ablation: replace the AllGather with 8 local slice
            # copies (wrong numerics, negligible DMA time)
            agin_t, agout_t = cc_tiles
            for j in range(cfg.NC):
                nc.gpsimd.dma_start(agout_t[P * j: P * (j + 1), :], agin_t[:])
        else:
            nc.gpsimd.collective_compute(kind, op, replica_groups=replica_groups,
                                         ins=ins, outs=outs)
    T, NE, DA, NC = cfg.T, cfg.NE, cfg.DA, cfg.NC
    TSL = cfg.TSL
    RG = [list(range(NC))]

    # x slice staged PRE-TRANSPOSED by the host: [NE, TSL+P] (halo in front)
    xs = nc.declare_dram_parameter("xs", [NE, TSL + P], BF16, isOutput=False)
    wk = nc.declare_dram_parameter("wk", [4 * P, cfg.NKT * 512], BF16, isOutput=False)
    wv = nc.declare_dram_parameter("wv", [4 * P, cfg.NKT * 512], BF16, isOutput=False)
    wr = nc.declare_dram_parameter("wr", [4 * P, cfg.NKT * 512],
                                   F8 if FP8_R else BF16, isOutput=False)
    wo = nc.declare_dram_parameter("wo", [cfg.NOT * P, cfg.NKT2 * 512], BF16, isOutput=False)
    tmk = nc.declare_dram_parameter("tmk", [P, cfg.NKT], F32, isOutput=False)
    tmdv = nc.declare_dram_parameter("tmdv", [P, cfg.NKT], F32, isOutput=False)
    tmdr = nc.declare_dram_parameter("tmdr", [P, cfg.NKT], F32, isOutput=False)
    lam = nc.declare_dram_parameter("lam", [P, cfg.NKT2], F32, isOutput=False)
    eu = nc.declare_dram_parameter("eu", [P, cfg.NKT2], F32, isOutput=False)
    l512 = nc.declare_dram_parameter("l512", [P, cfg.NKT2], F32, isOutput=False)
    ptab = nc.declare_dram_parameter("ptab", [P, cfg.NKT2 * TSL], BF16, isOutput=False)
    out = nc.declare_dram_parameter("out", [TSL, NE], F32, isOutput=True)

    with tile.TileContext(nc) as tc, ExitStack() as octx:
        dram = octx.enter_context(tc.tile_pool(name="dram", bufs=1, space="DRAM"))
        psum = octx.enter_context(tc.tile_pool(name="psum", bufs=8, space="PSUM"))
        const_pool = octx.enter_context(tc.tile_pool(name="const", bufs=1))
        tokp = octx.enter_context(tc.tile_pool(name="tokp", bufs=2))

        # small constants
        tm_sb = {}
        for name, src in (("k", tmk), ("dv", tmdv), ("dr", tmdr)):
            t = const_pool.tile([P, cfg.NKT], F32, tag=f"tm{name}", name=f"tm{name}_sb")
            nc.sync.dma_start(t[:], src[:])
            tm_sb[name] = t
        lam_sb = const_pool.tile([P, cfg.NKT2], F32, tag="lam")
        nc.sync.dma_start(lam_sb[:], lam[:])
        eu_sb = const_pool.tile([P, cfg.NKT2], F32, tag="eu")
        nc.sync.dma_start(eu_sb[:], eu[:])
        l512_sb = const_pool.tile([P, cfg.NKT2], F32, tag="l512")
        nc.sync.dma_start(l512_sb[:], l512[:])

        # DRAM bounce buffers for the carry AllGather (shared across reps)
        agin = dram.tile([P, 2 * cfg.NKT2], F32, tag="agin", name="agin")
        agout = dram.tile([NC * P, 2 * cfg.NKT2], F32, tag="agout", name="agout")

        prev_osts = None
        for rep in range(reps):
            prev_osts = _emit_body(
                nc, tc, cfg, rep, tm_sb, lam_sb, eu_sb, l512_sb,
                agin, agout, xs, wk, wv, wr, wo, ptab, out, psum,
                _collective, RG, tokp, prev_osts, ablate)

    nc.finalize()
    return nc


def _make_token(nc, tokp, osts, R):
    """Tiny persistent tile whose value depends on all final staging tiles —
    the next rep's gate reads it to serialize bodies for timing."""
    tok = tokp.tile([1, 8], bass.mybir.dt.float32, tag="tok", name=R + "tok")
    for i, o in enumerate(osts):
        nc.vector.tensor_copy(tok[0:1, 2 * (i % 4):2 * (i % 4) + 2],
                              o[0:1, 0:2])
    return tok


def _emit_body(nc, tc, cfg, rep, tm_sb, lam_sb, eu_sb, l512_sb,
               agin, agout, xs, wk, wv, wr, wo, ptab, out, psum,
               _collective, RG, tokp=None, prev_osts=None, ablate=None):
    T, NE, DA, NC = cfg.T, cfg.NE, cfg.DA, cfg.NC
    TSL = cfg.TSL
    XW = TSL + P
    W2 = 2 * cfg.NKT2
    R = f"r{rep}_"
    wdram = {"k": wk, "v": wv, "r": wr}
    HKT = cfg.NKT // 2            # kt tiles per weight half-strip
    mixes = {"k": [], "v": [], "r": []}

    def load_half(pool, name, s, half, queue):
        if name == "r" and FP8_R:
            wt = pool.tile([P, HKT * 512], F8, tag="wst8",
                           name=R + f"w_{name}_{s}_{half}")
        else:
            wt = pool.tile([P, HKT * 512], BF16, tag="wst",
                           name=R + f"w_{name}_{s}_{half}")
        queue.dma_start(
            wt[:], wdram[name][P * s: P * (s + 1),
                               HKT * 512 * half: HKT * 512 * (half + 1)])
        return wt

    def strip_mms(name, s, wts, slab, srb=None):
        """matmuls + drain for weight strip s (channel blocks 4s..4s+3)."""
        pts = [psum.tile([P, TSL], F32, tag="pp",
                         name=R + f"ps_{name}_{s}_{c4}")
               for c4 in range(4)]
        if name == "r" and FP8_R:
            NQ = cfg.NKT // 2
            HQ = NQ // 2
            for q in range(NQ):
                wt = wts[q // HQ][:, :]
                qo = q % HQ
                r_ = mixes["r"][q][:, :]
                rhs = bass.AP(r_.tensor, r_.offset,
                              [r_.ap[0], [TSL, 2], [1, TSL]])
                for c4 in range(4):
                    lhsT = bass.AP(wt.tensor,
                                   wt.offset + qo * 1024 + c4 * 256,
                                   [wt.ap[0], [128, 2], [1, 128]])
                    nc.tensor.matmul(
                        pts[c4][:], lhsT, rhs,
                        start=(q == 0), stop=(q == NQ - 1),
                        perf_mode=mybir.MatmulPerfMode.DoubleRow)
        else:
            for kt in range(cfg.NKT):
                wt = wts[kt // HKT]
                ko = kt % HKT
                for c4 in range(4):
                    nc.tensor.matmul(
                        pts[c4][:],
                        wt[:, ko * 512 + 128 * c4: ko * 512 + 128 * (c4 + 1)],
                        mixes[name][kt][:, :],
                        start=(kt == 0), stop=(kt == cfg.NKT - 1))
        if name == "r":
            for c4 in range(4):
                kt2 = 4 * s + c4
                nc.scalar.activation(srb[:, TSL * kt2: TSL * (kt2 + 1)],
                                     pts[c4][:], ACTF.Sigmoid,
                                     scale=(1.0 / WR_SCALE) if FP8_R else 1.0)
        else:
            for c4 in range(4):
                nc.scalar.copy(slab[:, TSL * c4: TSL * (c4 + 1)], pts[c4][:])

    # =========== emission ===========
    body_ctx = ExitStack()
    slabp = body_ctx.enter_context(tc.tile_pool(name=R + "slabp", bufs=1))
    scanp = body_ctx.enter_context(tc.tile_pool(name=R + "scanp", bufs=2))
    carryp = body_ctx.enter_context(tc.tile_pool(name=R + "carryp", bufs=1))
    srbp = body_ctx.enter_context(tc.tile_pool(name=R + "srbp", bufs=1))

    # sigmoid(r)^T, token-sharded, [128, TSL] per channel block kt2
    srb = srbp.tile([P, cfg.NKT2 * TSL], BF16, tag="srb", name=R + "srb")
    # per-strip slabs: become ek/ekv then denA/numA in place (live to y phase)
    kslab = [slabp.tile([P, 4 * TSL], BF16, tag=f"ks{s}", name=R + f"ks{s}")
             for s in range(4)]
    vslab = [slabp.tile([P, 4 * TSL], BF16, tag=f"vs{s}", name=R + f"vs{s}")
             for s in range(4)]
    # local scan end states (P,Q) for all 16 ptiles
    ends = carryp.tile([P, W2], F32, tag="ends", name=R + "ends")
    allends = carryp.tile([P, W2 * NC], F32, tag="allends", name=R + "allends")
    globc = carryp.tile([P, W2 * NC], F32, tag="globc", name=R + "globc")
    pprev = carryp.tile([P, W2], F32, tag="pprev", name=R + "pprev")
    zcol = carryp.tile([P, 1], F32, tag="zcol", name=R + "zcol")
    nc.gpsimd.memset(zcol[:], 0.0)

    def scan_unit(kt2):
        """local WKV scan + numA/denA for channel ptile kt2 (in place over
        the slab blocks: kslab -> denA, vslab -> numA).

        stt is pathologically slow on real DVE (~2us/[128,512] vs ~260ns
        for plain tensor_tensor); per-channel scales run on ACT (~300ns,
        scale accepts a [P,1] AP) and DVE keeps only muls/adds/scans."""
        s, c4 = kt2 // 4, kt2 % 4
        kblk = kslab[s][:, c4 * TSL: (c4 + 1) * TSL]
        vblk = vslab[s][:, c4 * TSL: (c4 + 1) * TSL]
        lam_b = _bcast(lam_sb[:, kt2:kt2 + 1], TSL)
        eu_ap = eu_sb[:, kt2:kt2 + 1]
        # k -> e^k (ACT, in place); v -> e^k * v (DVE, in place)
        nc.scalar.activation(kblk, kblk, ACTF.Exp)
        nc.vector.tensor_mul(vblk, kblk, vblk)
        Pst = scanp.tile([P, TSL + 1], BF16, tag=f"Pst{kt2 % 2}")
        Qst = scanp.tile([P, TSL + 1], BF16, tag=f"Qst{kt2 % 2}")
        nc.gpsimd.tensor_copy(Pst[:, 0:1], zcol[:])
        nc.gpsimd.tensor_copy(Qst[:, 0:1], zcol[:])
        nc.vector.tensor_tensor_scan(
            Pst[:, 1:TSL + 1], lam_b, vblk, Pst[:, 0:1], op0=AL.mult, op1=AL.add)
        nc.vector.tensor_tensor_scan(
            Qst[:, 1:TSL + 1], lam_b, kblk, Qst[:, 0:1], op0=AL.mult, op1=AL.add)
        # stage end states for the carry AllGather (ACT copy converts to f32)
        nc.scalar.copy(ends[:, 2 * kt2: 2 * kt2 + 1], Pst[:, TSL:TSL + 1])
        nc.scalar.copy(ends[:, 2 * kt2 + 1: 2 * kt2 + 2], Qst[:, TSL:TSL + 1])
        # numA = eu*ekv + p_shift ; denA = eu*ek + q_shift (in place):
        # ACT does the eu scale, DVE the add
        nc.scalar.activation(vblk, vblk, ACTF.Copy, scale=eu_ap)
        nc.vector.tensor_add(vblk, vblk, Pst[:, 0:TSL])
        nc.scalar.activation(kblk, kblk, ACTF.Copy, scale=eu_ap)
        nc.vector.tensor_add(kblk, kblk, Qst[:, 0:TSL])

    with tc.tile_pool(name=R + "mxp", bufs=1) as mxp, \
         tc.tile_pool(name=R + "wstp", bufs=2) as wstp, \
         tc.tile_pool(name=R + "wrp", bufs=2) as wrp:

        with tc.tile_pool(name=R + "dp", bufs=1) as dp:
            dts = []
            with tc.tile_pool(name=R + "xtp", bufs=2) as xtp:
                # transpose x slice in 4 chunk tiles (2 rotating buffers)
                NCH = 4
                ktc = cfg.NKT // NCH
                xtrc = []
                for c in range(NCH):
                    t = xtp.tile([P, ktc * XW], BF16, tag=f"xtr{c % 2}",
                                 name=R + f"xtr{c}")
                    xtrc.append(t)
                    if c == 0 and rep > 0:
                        nc.vector.tensor_copy(t[0:1, 0:8], prev_osts[0:1, 0:8])
                    for kt in range(ktc * c, ktc * (c + 1)):
                        o = (kt % ktc) * XW
                        nc.sync.dma_start(t[:, o: o + XW],
                                          xs[P * kt: P * (kt + 1), :])

                def xparts(kt):
                    t = xtrc[kt // ktc]
                    o = (kt % ktc) * XW
                    return t[:, o + P: o + XW], t[:, o + P - 1: o + XW - 1]

                # d = x - xx and the k-mix (only readers of the x chunks);
                # per-channel scales on ACT, adds on DVE (stt is slow)
                for kt in range(cfg.NKT):
                    xm, xx = xparts(kt)
                    d = dp.tile([P, TSL], BF16, tag=f"d{kt}", name=R + f"d{kt}")
                    nc.vector.tensor_sub(d[:], xm, xx)
                    dts.append(d)
                    mt_ = mxp.tile([P, TSL], BF16, tag=f"mxk{kt}",
                                   name=R + f"mxk{kt}")
                    nc.scalar.activation(mt_[:], d[:], ACTF.Copy,
                                         scale=tm_sb["k"][:, kt:kt + 1])
                    nc.vector.tensor_add(mt_[:], mt_[:], xx)
                    mixes["k"].append(mt_)

            # xv = xk + (tmv - tmk) * d ; xr = xk + (tmr - tmk) * d
            for kt in range(cfg.NKT):
                mt_ = mxp.tile([P, TSL], BF16, tag=f"mxv{kt}",
                               name=R + f"mxv{kt}")
                nc.scalar.activation(mt_[:], dts[kt][:], ACTF.Copy,
                                     scale=tm_sb["dv"][:, kt:kt + 1])
                nc.vector.tensor_add(mt_[:], mt_[:], mixes["k"][kt][:])
                mixes["v"].append(mt_)
            if FP8_R:
                for kt in range(cfg.NKT):
                    q, s2 = kt // 2, kt % 2
                    if s2 == 0:
                        t8 = mxp.tile([P, 2 * TSL], F8, tag=f"mxr{q}",
                                      name=R + f"mxr{q}")
                        mixes["r"].append(t8)
                    tr = dp.tile([P, TSL], BF16, tag=f"tr{kt % 2}",
                                 name=R + f"tr{kt}")
                    nc.scalar.activation(tr[:], dts[kt][:], ACTF.Copy,
                                         scale=tm_sb["dr"][:, kt:kt + 1])
                    nc.vector.tensor_add(
                        mixes["r"][q][:, s2 * TSL:(s2 + 1) * TSL],
                        tr[:], mixes["k"][kt][:])
            else:
                for kt in range(cfg.NKT):
                    mt_ = mxp.tile([P, TSL], BF16, tag=f"mxr{kt}",
                                   name=R + f"mxr{kt}")
                    nc.scalar.activation(mt_[:], dts[kt][:], ACTF.Copy,
                                         scale=tm_sb["dr"][:, kt:kt + 1])
                    nc.vector.tensor_add(mt_[:], mt_[:], mixes["k"][kt][:])
                    mixes["r"].append(mt_)

        # r strip 0 weights: load early from the sync queue
        wt_r0 = [load_half(wrp, "r", 0, hf, nc.sync) for hf in range(2)]

        # k / v projections, strip-interleaved so scans start early;
        # scan units emitted right after each v strip
        for s in range(4):
            wts_k = [load_half(wstp, "k", s, hf, nc.scalar) for hf in range(2)]
            strip_mms("k", s, wts_k, kslab[s])
            wts_v = [load_half(wstp, "v", s, hf, nc.scalar) for hf in range(2)]
            strip_mms("v", s, wts_v, vslab[s])
            if ablate != "noelem":
                for c4 in range(4):
                    scan_unit(4 * s + c4)

        # carry AllGather: ends -> DRAM -> collective -> allends
        if ablate == "noelem":
            nc.vector.memset(ends[:], 0.0)
        nc.sync.dma_start(agin[:], ends[:])
        _collective("AllGather", AL.bypass, replica_groups=RG,
                    ins=[agin[:].opt()], outs=[agout[:].opt()],
                    cc_tiles=(agin, agout))
        so = agout[:]
        src3 = bass.AP(so.tensor, so.offset, [[W2, P], [P * W2, NC], [1, W2]])
        nc.sync.dma_start(allends[:], src3)

        # r strips 1-3 weights then the r projection (token-sharded sigmoid)
        wt_r1 = [load_half(wrp, "r", 1, hf, nc.scalar) for hf in range(2)]
        wt_r2 = [load_half(wrp, "r", 2, hf, nc.scalar) for hf in range(2)]
        wt_r3 = [load_half(wrp, "r", 3, hf, nc.gpsimd) for hf in range(2)]
        for s, wts_r in ((0, wt_r0), (1, wt_r1), (2, wt_r2), (3, wt_r3)):
            strip_mms("r", s, wts_r, None, srb=srb)

    # -------- post-projection: carry combine + correction + y ------------
    atb_ctx = ExitStack()
    atbp = atb_ctx.enter_context(tc.tile_pool(name=R + "atbp", bufs=1))
    recp = atb_ctx.enter_context(tc.tile_pool(name=R + "recp", bufs=2))
    # y*sr (the output-matmul lhsT), [128, TSL] per kt2
    atb = atbp.tile([P, cfg.NKT2 * TSL], BF16, tag="atb", name=R + "atb")
    # lambda^n table, loaded late into the zone freed by the mix pools
    ptab_sb = atbp.tile([P, cfg.NKT2 * TSL], BF16, tag="ptab", name=R + "ptab")
    nc.sync.dma_start(ptab_sb[:], ptab[:])

    if ablate in ("noelem", "noy"):
        for kt2 in range(cfg.NKT2):
            nc.vector.tensor_copy(atb[:, kt2 * TSL: (kt2 + 1) * TSL],
                                  srb[:, kt2 * TSL: (kt2 + 1) * TSL])
    if ablate is None:
      # prefix-combine the carries (every core computes all prefixes):
      # glob_j = allends_j + lam512 * glob_{j-1}, vectorized over all 32
      # (ptile, P/Q) columns; lam512 pairs come from a 3D stride-0 AP
      l512bc = bass.AP(l512_sb.tensor, l512_sb[:].offset,
                       [l512_sb[:].ap[0], [1, cfg.NKT2], [0, 2]])
      nc.vector.tensor_copy(globc[:, 0:W2], allends[:, 0:W2])
      for j in range(1, NC):
        nc.vector.tensor_tensor(globc[:, W2 * j: W2 * (j + 1)],
                                globc[:, W2 * (j - 1): W2 * j],
                                l512bc, op=AL.mult)
        nc.vector.tensor_add(globc[:, W2 * j: W2 * (j + 1)],
                             globc[:, W2 * j: W2 * (j + 1)],
                             allends[:, W2 * j: W2 * (j + 1)])

      # select my predecessor's prefix
      idx = nc.vector.partition_id()
      for i in tc.Switch(idx, NC):
        if i == 0:
            nc.vector.memset(pprev[:], 0.0)
        else:
            nc.vector.tensor_copy(pprev[:], globc[:, W2 * (i - 1): W2 * i])

      # y phase per ptile: num = ptab*P_prev + numA (in place over vslab),
      # den likewise over kslab; recip; y*sr into atb.  ACT computes the
      # ptab*prev scale (scale AP), DVE the adds/recip/muls.
      for kt2 in range(cfg.NKT2):
        s, c4 = kt2 // 4, kt2 % 4
        kblk = kslab[s][:, c4 * TSL: (c4 + 1) * TSL]
        vblk = vslab[s][:, c4 * TSL: (c4 + 1) * TSL]
        pt_blk = ptab_sb[:, kt2 * TSL: (kt2 + 1) * TSL]
        corr = recp.tile([P, TSL], BF16, tag=f"cor{kt2 % 2}",
                         name=R + f"cor{kt2}")
        den = recp.tile([P, TSL], F32, tag=f"den{kt2 % 2}",
                        name=R + f"den{kt2}")
        nc.scalar.activation(corr[:], pt_blk, ACTF.Copy,
                             scale=pprev[:, 2 * kt2: 2 * kt2 + 1])
        nc.vector.tensor_add(vblk, vblk, corr[:])
        nc.scalar.activation(den[:], pt_blk, ACTF.Copy,
                             scale=pprev[:, 2 * kt2 + 1: 2 * kt2 + 2])
        nc.vector.tensor_add(den[:], den[:], kblk)
        nc.vector.reciprocal_approx_fast(den[:], den[:])
        nc.vector.tensor_mul(vblk, vblk, den[:])
        nc.vector.tensor_mul(atb[:, kt2 * TSL: (kt2 + 1) * TSL],
                             vblk, srb[:, kt2 * TSL: (kt2 + 1) * TSL])
      del idx

    # ---------------- phase C: output matmul -----------------------------
    with tc.tile_pool(name=R + "wop", bufs=4) as wop, \
         tc.tile_pool(name=R + "ostl", bufs=1) as ostl:
        wots = []
        for nt in range(cfg.NOT):
            wot = wop.tile([P, cfg.NKT2 * 512], BF16, tag="wo",
                           name=R + f"wo_{nt}")
            nc.scalar.dma_start(wot[:], wo[P * nt: P * (nt + 1), :])
            wots.append(wot)

        osts = [ostl.tile([P, NE], F32, tag=f"ost{mt}", name=R + f"ost{mt}")
                for mt in range(cfg.NMT)]
        for ntp in range(cfg.NOT // 2):
            nts = (2 * ntp, 2 * ntp + 1)
            pts = {(mt_, i_): psum.tile([P, 512], F32, tag="pp",
                                        name=R + f"po_{ntp}_{mt_}_{i_}")
                   for mt_ in range(cfg.NMT) for i_ in range(2)}
            for kt in range(cfg.NKT2):
                for mt in range(cfg.NMT):
                    lhsT = atb[:, kt * TSL + P * mt: kt * TSL + P * (mt + 1)]
                    for i_ in range(2):
                        nc.tensor.matmul(
                            pts[(mt, i_)][:], lhsT,
                            wots[nts[i_]][:, 512 * kt: 512 * (kt + 1)],
                            start=(kt == 0), stop=(kt == cfg.NKT2 - 1))
            for mt in range(cfg.NMT):
                for i_ in range(2):
                    nt = nts[i_]
                    nc.scalar.copy(osts[mt][:, 512 * nt: 512 * (nt + 1)],
                                   pts[(mt, i_)][:])
                nc.sync.dma_start(
                    out[P * mt: P * (mt + 1), 1024 * ntp: 1024 * (ntp + 1)],
                    osts[mt][:, 1024 * ntp: 1024 * (ntp + 1)])
        tok = _make_token(nc, tokp, osts, R)
    atb_ctx.close()
    body_ctx.close()
    return tok


# ------------------------------------------------------------------------
# host side
# ------------------------------------------------------------------------

_CACHE = {}


def _get_nc(cfg: Cfg):
    key = (cfg.T, cfg.NE, cfg.DA, cfg.NC)
    if key not in _CACHE:
        _CACHE[key] = build_kernel(cfg)
    return _CACHE[key]


def make_in_maps(cfg: Cfg, x, time_first, time_decay, time_mix_k, time_mix_v,
                 time_mix_r, W_key, W_value, W_receptance, W_output):
    T, NE, DA, NC = cfg.T, cfg.NE, cfg.DA, cfg.NC
    TSL = cfg.TSL
    bf = ml_dtypes.bfloat16

    x = np.asarray(x, np.float32)
    xpad = np.zeros((NE, P + T), bf)
    xpad[:, P:] = x.astype(bf).T

    def tile_w_strips(w):
        # [NE, DA] -> [4*P, NKT*512]: strip s row p, element kt*512+128*c4+m
        # = W[128kt+p, 128*(4s+c4)+m]
        w = np.asarray(w, np.float32).astype(bf)
        w4 = w.reshape(cfg.NKT, P, cfg.NKT2, P)   # [kt, p, kt2, c]
        outw = np.empty((4 * P, cfg.NKT * 512), bf)
        for s in range(4):
            blk = w4[:, :, 4 * s: 4 * (s + 1), :]   # [kt, p, 4, c]
            outw[P * s: P * (s + 1)] = (
                blk.transpose(1, 0, 2, 3).reshape(P, cfg.NKT * 512))
        return np.ascontiguousarray(outw)

    def tile_w(w, nkt, ng):
        # [DA, NE] -> [NG*P, NKT*512]: strip g rows hold W[128kt+p, 512g+c]
        w = np.asarray(w, np.float32).astype(bf)
        return np.ascontiguousarray(
            w.reshape(nkt, P, ng, 512).transpose(2, 1, 0, 3)
            .reshape(ng * P, nkt * 512))

    def tile_w_strips8(w, scale=WR_SCALE):
        # fp8 DoubleRow layout: strip s row p, element
        # q*1024 + c4*256 + s2*128 + m = W[256q + 128*s2 + p,
        #                                  128*(4s+c4) + m] * scale
        f8 = ml_dtypes.float8_e4m3
        w = np.asarray(w, np.float64) * scale
        w4 = np.clip(w, -240, 240).astype(np.float32).reshape(
            cfg.NKT, P, cfg.NKT2, P)
        outw = np.empty((4 * P, cfg.NKT * 512), f8)
        for s in range(4):
            blk = w4[:, :, 4 * s: 4 * (s + 1), :]        # [kt, p, c4, m]
            blk = blk.reshape(cfg.NKT // 2, 2, P, 4, P)  # [q, s2, p, c4, m]
            outw[P * s: P * (s + 1)] = (
                blk.transpose(2, 0, 3, 1, 4)
                .reshape(P, cfg.NKT * 512).astype(f8))
        return np.ascontiguousarray(outw)

    wk16 = tile_w_strips(W_key)
    wv16 = tile_w_strips(W_value)
    wr16 = tile_w_strips8(W_receptance) if FP8_R else tile_w_strips(W_receptance)
    wo16 = tile_w(W_output, cfg.NKT2, cfg.NOT)

    def col_fold(v, n_t):  # [n_t*P] -> [P, n_t]
        return np.ascontiguousarray(
            np.asarray(v, np.float64).reshape(-1)[: n_t * P]
            .reshape(n_t, P).T.astype(np.float32))

    tmk_v = np.asarray(time_mix_k, np.float64).reshape(-1)
    tmv_v = np.asarray(time_mix_v, np.float64).reshape(-1)
    tmr_v = np.asarray(time_mix_r, np.float64).reshape(-1)
    tmk_a = col_fold(tmk_v, cfg.NKT)
    tmdv_a = col_fold(tmv_v - tmk_v, cfg.NKT)
    tmdr_a = col_fold(tmr_v - tmk_v, cfg.NKT)

    td = np.asarray(time_decay, np.float64).reshape(-1)
    lam_full = np.exp(-np.exp(td))                       # [DA]
    eu_full = np.exp(np.asarray(time_first, np.float64).reshape(-1))

    lam_a = col_fold(lam_full, cfg.NKT2)
    eu_a = col_fold(eu_full, cfg.NKT2)
    l512_a = col_fold(lam_full ** TSL, cfg.NKT2)
    # powtab[p, 512*pt + n] = lam[128pt+p]^n
    n_idx = np.arange(TSL, dtype=np.float64)
    lam_fold = lam_full.reshape(cfg.NKT2, P)             # [pt, p]
    ptab_a = np.ascontiguousarray(
        (lam_fold[:, :, None] ** n_idx[None, None, :])   # [pt, p, n]
        .transpose(1, 0, 2).reshape(P, cfg.NKT2 * TSL).astype(bf))

    in_maps = []
    for i in range(NC):
        xsl = np.ascontiguousarray(xpad[:, TSL * i: TSL * i + TSL + P])
        in_maps.append({
            "xs": xsl, "wk": wk16, "wv": wv16, "wr": wr16, "wo": wo16,
            "tmk": tmk_a, "tmdv": tmdv_a, "tmdr": tmdr_a,
            "lam": lam_a, "eu": eu_a, "l512": l512_a, "ptab": ptab_a,
        })
    return in_maps


def kernel(x, time_first, time_decay, time_mix_k, time_mix_v, time_mix_r,
           W_key, W_value, W_receptance, W_output, _trace=False):
    cfg = Cfg(T=int(np.asarray(x).shape[0]), NE=int(np.asarray(x).shape[1]),
              DA=int(np.asarray(time_decay).reshape(-1).shape[0]), NC=8)
    nc = _get_nc(cfg)
    in_maps = make_in_maps(cfg, x, time_first, time_decay, time_mix_k,
                           time_mix_v, time_mix_r, W_key, W_value,
                           W_receptance, W_output)
    res = run_bass_kernel_spmd(nc, in_maps, core_ids=list(range(cfg.NC)),
                               trace=_trace)
    outp = np.concatenate([res.results[i]["out"] for i in range(cfg.NC)], axis=0)
    out_final = outp.astype(np.float32)
    if _trace:
        return out_final, res
    return out_final
